# revision 3
# baseline (speedup 1.0000x reference)
"""CustomGRU kernel for Trainium2 — 8-core data-parallel over batch.

Reference computation (per batch row b):
    h_0 = 0
    for t in 0..T-1:
        z = sigmoid([h, x_t] @ Wz + bz)
        r = sigmoid([h, x_t] @ Wr + br)
        hh = tanh([r*h, x_t] @ Wh + bh)
        h = (1-z)*h + z*hh
    out = h @ Wo + bo

Strategy:
  - Shard batch (1024) over 8 cores -> 128 rows/core.
  - State kept transposed in SBUF: hT [H=128 partitions, B=128 free].
  - Recurrent matmuls: lhsT = Wg[0:H,:] (stationary), rhs = hT.
  - x-projections: x is pre-transposed host-side to [T, 17, B] tiles
    (16 features + a ones-row so the gate bias folds into the weights),
    grouped in 32-partition quarters so K=17 matmuls hit 32-aligned
    row groups. Accumulated into the same PSUM region as the recurrent
    matmul (start=True then start=False).
"""

import numpy as np

import concourse.bacc as bacc
import concourse.bass as bass
import concourse.mybir as mybir
from concourse.bass_utils import run_bass_kernel_spmd
from concourse.tile import TileContext

B, T, I, H, O = 1024, 4096, 16, 128, 8
N_CORES = 8
BC = B // N_CORES  # batch rows per core

F32 = mybir.dt.float32
AF = mybir.ActivationFunctionType


def build_gru_nc(t_len: int, tc_chunk: int, dtype=F32):
    """Emit the Bass module for a GRU over t_len steps, x chunked tc_chunk steps."""
    nchunk = t_len // tc_chunk
    qt = tc_chunk // 4  # steps per 32-partition quarter
    nc = bacc.Bacc("TRN2", target_bir_lowering=False, debug=False, num_devices=N_CORES)

    xt = nc.dram_tensor(
        "xt", [nchunk, 4, 17, qt * BC], F32, kind="ExternalInput"
    )
    wh = nc.dram_tensor("wh", [3, H, H], F32, kind="ExternalInput")
    wx17 = nc.dram_tensor("wx17", [17, 3 * H], F32, kind="ExternalInput")
    wo = nc.dram_tensor("wo", [H, O], F32, kind="ExternalInput")
    bo = nc.dram_tensor("bo", [O, 1], F32, kind="ExternalInput")
    out = nc.dram_tensor("out", [O, BC], F32, kind="ExternalOutput")

    with TileContext(nc) as tc:
        with (
            tc.tile_pool(name="const", bufs=1) as const,
            tc.tile_pool(name="xpool", bufs=2) as xpool,
            tc.tile_pool(name="state", bufs=1) as state,
            tc.tile_pool(name="work", bufs=2) as work,
            tc.tile_pool(name="psum", bufs=2, space="PSUM") as psum,
        ):
            # --- resident constants ---
            w_zh = const.tile([H, H], dtype, tag="wzh")
            w_rh = const.tile([H, H], dtype, tag="wrh")
            w_hh = const.tile([H, H], dtype, tag="whh")
            for g, wt in enumerate((w_zh, w_rh, w_hh)):
                nc.sync.dma_start(out=wt, in_=wh[g])
            wx_sb = const.tile([128, 3 * H], dtype, tag="wx")
            for q in range(4):
                nc.sync.dma_start(out=wx_sb[32 * q : 32 * q + 17, :], in_=wx17[:, :])
            wo_sb = const.tile([H, O], dtype, tag="wo")
            nc.sync.dma_start(out=wo_sb, in_=wo[:, :])
            bo_sb = const.tile([O, 1], F32, tag="bo")
            nc.sync.dma_start(out=bo_sb, in_=bo[:, :])

            h = state.tile([H, BC], dtype, tag="h")
            nc.vector.memset(h, 0.0)

            for ci in range(nchunk):
                xq = xpool.tile([128, qt * BC], dtype, tag="xq")
                for q in range(4):
                    nc.sync.dma_start(
                        out=xq[32 * q : 32 * q + 17, :], in_=xt[ci, q]
                    )
                for s in range(tc_chunk):
                    q, j = divmod(s, qt)
                    rx = xq[32 * q : 32 * q + 17, j * BC : (j + 1) * BC]
                    tp = (32 * q, 0)
                    pz = psum.tile([H, 2 * BC], F32, tag="zr")
                    nc.tensor.matmul(
                        pz[:, 0:BC], wx_sb[32 * q : 32 * q + 17, 0:H], rx,
                        start=True, stop=False, tile_position=tp,
                    )
                    nc.tensor.matmul(
                        pz[:, BC : 2 * BC], wx_sb[32 * q : 32 * q + 17, H : 2 * H], rx,
                        start=False, stop=False, tile_position=tp,
                        skip_group_check=True,
                    )
                    nc.tensor.matmul(
                        pz[:, 0:BC], w_zh, h, start=False, stop=False,
                        skip_group_check=True,
                    )
                    nc.tensor.matmul(
                        pz[:, BC : 2 * BC], w_rh, h, start=False, stop=True,
                        skip_group_check=True,
                    )
                    szr = work.tile([H, 2 * BC], dtype, tag="szr")
                    nc.scalar.activation(szr, pz, AF.Sigmoid)
                    rh = work.tile([H, BC], dtype, tag="rh")
                    nc.vector.tensor_mul(rh, szr[:, BC : 2 * BC], h)
                    pc = psum.tile([H, BC], F32, tag="c")
                    nc.tensor.matmul(
                        pc, wx_sb[32 * q : 32 * q + 17, 2 * H : 3 * H], rx,
                        start=True, stop=False, tile_position=tp,
                    )
                    nc.tensor.matmul(pc, w_hh, rh, start=False, stop=True)
                    th = work.tile([H, BC], dtype, tag="th")
                    nc.scalar.activation(th, pc, AF.Tanh)
                    d = work.tile([H, BC], dtype, tag="d")
                    nc.vector.tensor_sub(d, th, h)
                    e = work.tile([H, BC], dtype, tag="e")
                    nc.vector.tensor_mul(e, szr[:, 0:BC], d)
                    nc.vector.tensor_add(h, h, e)

            po = psum.tile([O, BC], F32, tag="o")
            nc.tensor.matmul(po, wo_sb, h, start=True, stop=True)
            osb = work.tile([O, BC], F32, tag="osb")
            nc.vector.tensor_scalar_add(osb, po, bo_sb[:, 0:1])
            nc.sync.dma_start(out=out[:, :], in_=osb)

    nc.finalize()
    return nc


def prep_inputs(x, Wz, bz, Wr, br, Wh, bh, Wo, bo, t_len, tc_chunk):
    """Host-side sharding + layout prep. Returns per-core input maps."""
    qt = tc_chunk // 4
    nchunk = t_len // tc_chunk
    wh_np = np.ascontiguousarray(np.stack([Wz[:H], Wr[:H], Wh[:H]]), np.float32)
    wx17_np = np.concatenate(
        [
            np.concatenate([Wg[H:], bg[None, :]], axis=0)
            for Wg, bg in ((Wz, bz), (Wr, br), (Wh, bh))
        ],
        axis=1,
    )
    wx17_np = np.ascontiguousarray(wx17_np, np.float32)  # [17, 3H]
    wo_np = np.ascontiguousarray(Wo, np.float32)
    bo_np = np.ascontiguousarray(bo.reshape(O, 1), np.float32)

    in_maps = []
    for c in range(N_CORES):
        xc = x[c * BC : (c + 1) * BC, :t_len]  # [BC, t_len, I]
        xtr = np.transpose(xc, (1, 2, 0))  # [t_len, I, BC]
        ones = np.ones((t_len, 1, BC), np.float32)
        x17 = np.concatenate([xtr, ones], axis=1)  # [t_len, 17, BC]
        x17 = x17.reshape(nchunk, 4, qt, 17, BC).transpose(0, 1, 3, 2, 4)
        x17 = np.ascontiguousarray(x17.reshape(nchunk, 4, 17, qt * BC), np.float32)
        in_maps.append(
            {"xt": x17, "wh": wh_np, "wx17": wx17_np, "wo": wo_np, "bo": bo_np}
        )
    return in_maps


_NC_CACHE: dict = {}


def run_gru(x, Wz, bz, Wr, br, Wh, bh, Wo, bo, t_len=T, tc_chunk=64, trace=False):
    key = (t_len, tc_chunk)
    if key not in _NC_CACHE:
        _NC_CACHE[key] = build_gru_nc(t_len, tc_chunk)
    nc = _NC_CACHE[key]
    in_maps = prep_inputs(x, Wz, bz, Wr, br, Wh, bh, Wo, bo, t_len, tc_chunk)
    res = run_bass_kernel_spmd(
        nc, in_maps, core_ids=list(range(N_CORES)), trace=trace
    )
    outs = [res.results[c]["out"].T for c in range(N_CORES)]  # each [BC, O]
    full = np.concatenate(outs, axis=0).astype(np.float32)
    return full, res


def kernel(x, Wz, bz, Wr, br, Wh, bh, Wo, bo):
    full, _ = run_gru(x, Wz, bz, Wr, br, Wh, bh, Wo, bo)
    return full
